# revision 6
# baseline (speedup 1.0000x reference)
# Multi-head attention (B=2, S=2048, D=1024, H=16, dh=64) on 8 TRN2 NeuronCores.
# Sharding: core = batch * 4 + head_group; each core handles one batch and 4 heads.
# v2: k-strip software pipeline. All Q projections run up front; per k-strip t
# the kernel computes K/V projections, then scores+exp+PV for every q-tile
# j >= t over that strip's 4 k-tiles, accumulating unnormalized attention and
# softmax denominators (ones-augmented V, M=65) in SBUF across strips. K/V
# projections for strip t+1 and Wo output-projection chunks are interleaved as
# PE filler between the exp-dependent PV matmuls, keeping TensorE continuously
# busy (HAM stays at 8/8) while ScalarE's exp stream hides underneath.
# Mask multiplies, V/pv1 evacuations go to GpSimd; Wo results DMA directly
# from PSUM; normalization is deferred and fused into the att evacuation.
import numpy as np
import ml_dtypes

from collections import deque

import concourse.bass as bass
import concourse.tile as tile
from concourse import bacc, mybir
from concourse import bass_utils

B, S, D = 2, 2048, 1024
H, DH = 16, 64
NCORES = 8
GROUPS = 4            # head groups per batch (cores per batch)
HPG = 4               # heads per group
FPG = HPG * DH        # 256 features per group
SQ_T, SK_T = 512, 128
NSQ, NSK = S // SQ_T, S // SK_T
NCH = D // 128        # 8 contraction chunks of d_model
BF16 = ml_dtypes.bfloat16

_BUILT = {}


def _classify(mask):
    """Verify the causal structure and pack diagonal keep tiles.
    Tile (i, j) covers k in [i*128, (i+1)*128), q in [j*512, (j+1)*512)."""
    keep_t = (~np.asarray(mask, dtype=bool)).T  # [k, q], True = attend
    ptiles = []
    pidx = {}
    for j in range(NSQ):
        for i in range(NSK):
            sub = keep_t[i * SK_T:(i + 1) * SK_T, j * SQ_T:(j + 1) * SQ_T]
            if i >= 4 * (j + 1):
                assert not sub.any()
            elif i // 4 == j:  # diagonal strip: partial tile
                pidx[(i, j)] = len(ptiles)
                ptiles.append(np.ascontiguousarray(sub.astype(BF16)))
            else:
                assert sub.all()
    return pidx, ptiles


def _build(n_ptiles):
    nc = bacc.Bacc("TRN2", target_bir_lowering=False, debug=False)
    dt = mybir.dt
    f32, bf = dt.float32, dt.bfloat16
    EXP = mybir.ActivationFunctionType.Exp
    MUL = mybir.AluOpType.mult
    ADD = mybir.AluOpType.add

    xq = nc.dram_tensor("xqt", [D, S], bf, kind="ExternalInput").ap()
    xk = nc.dram_tensor("xkt", [D, S], bf, kind="ExternalInput").ap()
    xv = nc.dram_tensor("xvt", [D, S], bf, kind="ExternalInput").ap()
    wq = nc.dram_tensor("wqt", [D, FPG], bf, kind="ExternalInput").ap()
    wk = nc.dram_tensor("wkt", [D, FPG], bf, kind="ExternalInput").ap()
    wv = nc.dram_tensor("wvt", [D, FPG], bf, kind="ExternalInput").ap()
    wo = nc.dram_tensor("wot", [FPG, D], bf, kind="ExternalInput").ap()
    kp = nc.dram_tensor("keep", [max(n_ptiles, 1) * SK_T, SQ_T], bf,
                        kind="ExternalInput").ap()
    out = nc.dram_tensor("out", [S, D], f32, kind="ExternalOutput").ap()

    xq_v = xq.rearrange("(c p) s -> p c s", p=128)
    xk_v = xk.rearrange("(c p) s -> p c s", p=128)
    xv_v = xv.rearrange("(c p) s -> p c s", p=128)
    wq_v = wq.rearrange("(c p) f -> p c f", p=128)
    wk_v = wk.rearrange("(c p) f -> p c f", p=128)
    wv_v = wv.rearrange("(c p) f -> p c f", p=128)
    wo_v = wo.rearrange("(c p) o -> p c o", p=128)
    kp_v = kp.rearrange("(n p) s -> p n s", p=128)
    out_v = out.rearrange("(r p) o -> r p o", p=128)

    with tile.TileContext(nc) as tc:
        with (
            tc.tile_pool(name="consts", bufs=1) as consts,
            tc.tile_pool(name="x", bufs=2) as xpool,
            tc.tile_pool(name="sc", bufs=2, space="PSUM") as sc_ps,
            tc.tile_pool(name="pv", bufs=2, space="PSUM") as pv_ps,
            tc.tile_pool(name="aux", bufs=2, space="PSUM") as aux_ps,
            tc.tile_pool(name="work", bufs=4) as work,
            tc.tile_pool(name="probs", bufs=6) as prpool,
        ):
            # ---------------- SBUF constants ----------------
            wq_sb = consts.tile([128, NCH, FPG], bf)
            wk_sb = consts.tile([128, NCH, FPG], bf)
            wv_sb = consts.tile([128, NCH, FPG], bf)
            wo_sb = consts.tile([128, FPG // 128, D], bf)
            keep_sb = consts.tile([128, max(n_ptiles, 1), SQ_T], bf)
            ones_sb = consts.tile([128, 128], bf)
            xq_sb = consts.tile([128, NCH, S], bf)       # full Q input
            qh_sb = consts.tile([128, 2, S], bf)
            kh_sb = consts.tile([128, 2, S], bf)
            vh_sb = consts.tile([128, NSK, HPG, DH + 1], bf)
            att_acc = consts.tile([128, 2, S], f32)      # unnormalized att
            att_sb = consts.tile([128, 2, S], bf)        # normalized (Wo in)
            l_acc = consts.tile([128, NSQ, SQ_T], f32)   # denominators
            r4f = consts.tile([128, SQ_T], f32)
            r4 = consts.tile([128, SQ_T], bf)

            # ---------------- initial DMAs (minimal working set first) ----
            nc.sync.dma_start(wq_sb[:, 0:4, :], wq_v[:, 0:4, :])
            nc.sync.dma_start(wq_sb[:, 4:8, :], wq_v[:, 4:8, :])
            nc.sync.dma_start(xq_sb[:, :, 0:SQ_T], xq_v[:, :, 0:SQ_T])
            nc.scalar.dma_start(wk_sb[:, 0:4, :], wk_v[:, 0:4, :])
            nc.scalar.dma_start(wk_sb[:, 4:8, :], wk_v[:, 4:8, :])
            for t in range(1, NSQ):
                nc.sync.dma_start(xq_sb[:, :, bass.ts(t, SQ_T)],
                                  xq_v[:, :, bass.ts(t, SQ_T)])
            nc.scalar.dma_start(wv_sb[:], wv_v[:])
            if n_ptiles:
                nc.scalar.dma_start(keep_sb[:, 0:n_ptiles, :],
                                    kp_v[:, 0:n_ptiles, :])
            nc.scalar.dma_start(wo_sb[:], wo_v[:])

            nc.vector.memset(ones_sb[:], 1.0)
            nc.vector.memset(vh_sb[:], 1.0)  # ones column (col DH) survives
            nc.gpsimd.memset(l_acc[:], 1.0)  # untouched partitions -> r = 1
            # warm up the ACT exp table under the projection phase
            warm = work.tile([1, 8], f32, tag="warm")
            nc.scalar.activation(warm[:], ones_sb[0:1, 0:8], EXP, scale=0.0)

            xk_t, xv_t = {}, {}

            def dma_kv(t):
                sl = bass.ts(t, SQ_T)
                xkt = xpool.tile([128, NCH, SQ_T], bf, tag="xk")
                nc.scalar.dma_start(xkt[:], xk_v[:, :, sl])
                xvt = xpool.tile([128, NCH, SQ_T], bf, tag="xv")
                nc.scalar.dma_start(xvt[:], xv_v[:, :, sl])
                xk_t[t], xv_t[t] = xkt, xvt

            # ---------------- projection closures ----------------
            def qk_closures(t, wsb, xin_f, hout, full_x):
                """4 closures: (hp=0 c0-3, hp=0 c4-7+evac, hp=1 ...).
                full_x: xin_f() spans all of S (slice by t); else one tile."""
                sl = bass.ts(t, SQ_T)
                cl = []
                for hp in range(2):
                    hsl = bass.ts(hp, 128)
                    box = {}

                    def xsl(xin, c):
                        return xin[:, c, sl] if full_x else xin[:, c, :]

                    def first(hsl=hsl, box=box, wsb=wsb, xin_f=xin_f,
                              xsl=xsl):
                        ps = aux_ps.tile([128, SQ_T], f32, tag="aux")
                        xin = xin_f()
                        for c in range(4):
                            nc.tensor.matmul(ps[:], wsb[:, c, hsl],
                                             xsl(xin, c),
                                             start=(c == 0), stop=False)
                        box["ps"] = ps

                    def second(hp=hp, hsl=hsl, box=box, sl=sl, wsb=wsb,
                               xin_f=xin_f, hout=hout, xsl=xsl):
                        ps = box["ps"]
                        xin = xin_f()
                        for c in range(4, NCH):
                            nc.tensor.matmul(ps[:], wsb[:, c, hsl],
                                             xsl(xin, c),
                                             start=False, stop=(c == NCH - 1))
                        nc.vector.tensor_copy(hout[:, hp, sl], ps[:])

                    cl += [first, second]
                return cl

            def v_closures(t):
                """8 closures: per s4 subtile (c0-3, c4-7+evac)."""
                cl = []
                for s4 in range(SQ_T // SK_T):
                    i = t * (SQ_T // SK_T) + s4
                    box = {}

                    def vfirst(s4=s4, box=box, t=t):
                        ps = aux_ps.tile([128, SQ_T], f32, tag="aux")
                        for c in range(4):
                            nc.tensor.matmul(ps[:, 0:FPG],
                                             xv_t[t][:, c, bass.ts(s4, SK_T)],
                                             wv_sb[:, c, :],
                                             start=(c == 0), stop=False)
                        box["ps"] = ps

                    def vsecond(s4=s4, box=box, t=t, i=i):
                        ps = box["ps"]
                        for c in range(4, NCH):
                            nc.tensor.matmul(ps[:, 0:FPG],
                                             xv_t[t][:, c, bass.ts(s4, SK_T)],
                                             wv_sb[:, c, :],
                                             start=False, stop=(c == NCH - 1))
                        nc.vector.tensor_copy(
                            vh_sb[:, i, :, 0:DH],
                            ps[:, 0:FPG].rearrange("p (h d) -> p h d", h=HPG))

                    cl += [vfirst, vsecond]
                return cl

            def wo_closures(j):
                """8 closures: per (t4, o) a 2-MM accumulation + psum DMA."""
                cl = []
                for t4 in range(SQ_T // 128):
                    r_ = j * (SQ_T // 128) + t4
                    tsl = bass.ds(j * SQ_T + t4 * 128, 128)
                    for o in range(2):
                        def wone(r_=r_, tsl=tsl, o=o):
                            po = aux_ps.tile([128, SQ_T], f32, tag="aux")
                            for hp in range(2):
                                nc.tensor.matmul(po[:], att_sb[:, hp, tsl],
                                                 wo_sb[:, hp, bass.ts(o, 512)],
                                                 start=(hp == 0),
                                                 stop=(hp == 1))
                            ost = work.tile([128, 512], f32, tag="ost")
                            nc.vector.tensor_copy(ost[:], po[:])
                            nc.sync.dma_start(out_v[r_, :, bass.ts(o, 512)],
                                              ost[:])
                        cl.append(wone)
                return cl

            # ---------------- filler machinery ----------------
            fillers = deque()  # (tag, closure)

            def drain_tag(tag):
                keep = deque()
                while fillers:
                    tg, fn = fillers.popleft()
                    if tg == tag:
                        fn()
                    else:
                        keep.append((tg, fn))
                fillers.extend(keep)

            def pop_fillers(n):
                for _ in range(n):
                    if not fillers:
                        return
                    _, fn = fillers.popleft()
                    fn()

            # ---------------- prologue: all Q proj + K/V strip 0 ----------
            dma_kv(0)
            for t in range(NSQ):
                for fn in qk_closures(t, wq_sb, lambda: xq_sb, qh_sb, True):
                    fn()
            for fn in qk_closures(0, wk_sb, lambda: xk_t[0], kh_sb, False):
                fn()
            for fn in v_closures(0):
                fn()

            # ---------------- k-strips ----------------
            for t in range(NSQ):
                if t + 1 < NSQ:
                    dma_kv(t + 1)
                    for fn in qk_closures(t + 1, wk_sb,
                                          (lambda tt: lambda: xk_t[tt])(t + 1),
                                          kh_sb, False):
                        fillers.append((("kv", t + 1), fn))
                    for fn in v_closures(t + 1):
                        fillers.append((("kv", t + 1), fn))
                drain_tag(("kv", t))  # ensure this strip's K/V emitted
                positions = (NSQ - t) * 2 * 4
                for j in range(t, NSQ):
                    jsl = bass.ts(j, SQ_T)
                    diag = (j == t)
                    for hp in range(2):
                        pv0 = pv_ps.tile([DH + 1, SQ_T], f32, tag="pv")
                        pv1 = pv_ps.tile([DH + 1, SQ_T], f32, tag="pv")
                        for n in range(4):
                            i = 4 * t + n
                            isl = bass.ts(i, SK_T)
                            c0 = n * SK_T if diag else 0
                            qsl = bass.ds(j * SQ_T + c0, SQ_T - c0)
                            sc = sc_ps.tile([128, 2, SQ_T], f32, tag="sc")
                            nc.tensor.matmul(sc[:, 0, c0:SQ_T],
                                             kh_sb[0:64, hp, isl],
                                             qh_sb[0:64, hp, qsl], start=True,
                                             stop=True, tile_position=(0, 0))
                            nc.tensor.matmul(sc[:, 1, c0:SQ_T],
                                             kh_sb[64:128, hp, isl],
                                             qh_sb[64:128, hp, qsl],
                                             start=True, stop=True,
                                             tile_position=(64, 0))
                            pr = prpool.tile([128, 2, SQ_T], bf, tag="probs")
                            nc.scalar.activation(pr[:, :, c0:SQ_T],
                                                 sc[:, :, c0:SQ_T],
                                                 EXP, scale=0.125)
                            if diag:
                                kc = pidx_map[(i, j)]
                                nc.gpsimd.tensor_mul(pr[:, 0, c0:SQ_T],
                                                     pr[:, 0, c0:SQ_T],
                                                     keep_sb[:, kc, c0:SQ_T])
                                nc.gpsimd.tensor_mul(pr[:, 1, c0:SQ_T],
                                                     pr[:, 1, c0:SQ_T],
                                                     keep_sb[:, kc, c0:SQ_T])
                            # paced PE filler between exp and its PV consumer
                            nfill = -(-len(fillers) // max(positions, 1))
                            pop_fillers(min(nfill, 2))
                            positions -= 1
                            nc.tensor.matmul(pv0[:, c0:SQ_T],
                                             vh_sb[:, i, 2 * hp + 0, :],
                                             pr[:, 0, c0:SQ_T],
                                             start=(n == 0), stop=(n == 3))
                            nc.tensor.matmul(pv1[:, c0:SQ_T],
                                             vh_sb[:, i, 2 * hp + 1, :],
                                             pr[:, 1, c0:SQ_T],
                                             start=(n == 0), stop=(n == 3))
                        # evacuate psum: accumulate att + denominators in SBUF
                        p0 = 64 * hp
                        if t == 0:
                            nc.vector.tensor_copy(att_acc[0:64, hp, jsl],
                                                  pv0[0:64, :])
                            nc.vector.tensor_copy(l_acc[p0:p0 + 1, j, :],
                                                  pv0[DH:DH + 1, :])
                            nc.vector.tensor_copy(att_acc[64:128, hp, jsl],
                                                  pv1[0:64, :])
                            nc.vector.tensor_copy(l_acc[p0 + 32:p0 + 33, j, :],
                                                  pv1[DH:DH + 1, :])
                        else:
                            nc.vector.tensor_tensor(att_acc[0:64, hp, jsl],
                                                    pv0[0:64, :],
                                                    att_acc[0:64, hp, jsl],
                                                    ADD)
                            nc.vector.tensor_tensor(l_acc[p0:p0 + 1, j, :],
                                                    pv0[DH:DH + 1, :],
                                                    l_acc[p0:p0 + 1, j, :],
                                                    ADD)
                            nc.vector.tensor_tensor(att_acc[64:128, hp, jsl],
                                                    pv1[0:64, :],
                                                    att_acc[64:128, hp, jsl],
                                                    ADD)
                            nc.vector.tensor_tensor(
                                l_acc[p0 + 32:p0 + 33, j, :],
                                pv1[DH:DH + 1, :],
                                l_acc[p0 + 32:p0 + 33, j, :], ADD)
                    if diag:
                        # j is complete: normalize and queue its Wo chunks
                        nc.vector.reciprocal_approx_fast(r4f[:],
                                                         l_acc[:, j, :])
                        nc.vector.tensor_copy(r4[:], r4f[:])
                        for hp in range(2):
                            rb = aux_ps.tile([128, SQ_T], f32, tag="aux")
                            pe, po_ = 64 * hp, 64 * hp + 32
                            nc.tensor.matmul(rb[0:64, :],
                                             ones_sb[pe:pe + 1, 0:64],
                                             r4[pe:pe + 1, :], start=True,
                                             stop=True, tile_position=(pe, 0))
                            nc.tensor.matmul(rb[64:128, :],
                                             ones_sb[po_:po_ + 1, 64:128],
                                             r4[po_:po_ + 1, :], start=True,
                                             stop=True,
                                             tile_position=(po_, 64))
                            nc.vector.tensor_tensor(att_sb[:, hp, jsl],
                                                    att_acc[:, hp, jsl],
                                                    rb[:], MUL)
                        for fn in wo_closures(j):
                            fillers.append((("wo", j), fn))
            while fillers:
                _, fn = fillers.popleft()
                fn()

    nc.compile()
    return nc


pidx_map = None  # set per-build; mask layout is fixed (causal)


def _get_nc(mask):
    global pidx_map
    key = hash(np.asarray(mask, dtype=bool).tobytes())
    if key not in _BUILT:
        pidx, ptiles = _classify(mask)
        pidx_map = pidx
        _BUILT[key] = (_build(len(ptiles)), pidx, ptiles)
    pidx_map = _BUILT[key][1]
    return _BUILT[key]


def _kernel_impl(q, k, v, attn_mask, Wq, Wk, Wv, Wo, trace=False):
    q = np.asarray(q, dtype=np.float32)
    k = np.asarray(k, dtype=np.float32)
    v = np.asarray(v, dtype=np.float32)
    nc, pidx, ptiles = _get_nc(attn_mask)

    if ptiles:
        keep_packed = np.concatenate(ptiles, axis=0)
    else:
        keep_packed = np.zeros((SK_T, SQ_T), dtype=BF16)

    xt = {}
    for b in range(B):
        xt[("q", b)] = np.ascontiguousarray(q[b].T.astype(BF16))
        xt[("k", b)] = np.ascontiguousarray(k[b].T.astype(BF16))
        xt[("v", b)] = np.ascontiguousarray(v[b].T.astype(BF16))
    wslices = {}
    for g in range(GROUPS):
        fsl = slice(g * FPG, (g + 1) * FPG)
        wslices[("wq", g)] = np.ascontiguousarray(Wq[fsl, :].T.astype(BF16))
        wslices[("wk", g)] = np.ascontiguousarray(Wk[fsl, :].T.astype(BF16))
        wslices[("wv", g)] = np.ascontiguousarray(Wv[fsl, :].T.astype(BF16))
        wslices[("wo", g)] = np.ascontiguousarray(Wo[:, fsl].T.astype(BF16))

    in_maps = []
    for core in range(NCORES):
        b, g = core // GROUPS, core % GROUPS
        in_maps.append({
            "xqt": xt[("q", b)], "xkt": xt[("k", b)], "xvt": xt[("v", b)],
            "wqt": wslices[("wq", g)], "wkt": wslices[("wk", g)],
            "wvt": wslices[("wv", g)], "wot": wslices[("wo", g)],
            "keep": keep_packed,
        })

    res = bass_utils.run_bass_kernel_spmd(
        nc, in_maps, core_ids=list(range(NCORES)), trace=trace)

    out = np.zeros((B, S, D), dtype=np.float32)
    for core in range(NCORES):
        out[core // GROUPS] += res.results[core]["out"]
    return out, res


def kernel(q, k, v, attn_mask, Wq, Wk, Wv, Wo):
    out, _ = _kernel_impl(q, k, v, attn_mask, Wq, Wk, Wv, Wo)
    return out


# revision 7
# speedup vs baseline: 1.2334x; 1.2334x over previous
# Multi-head attention (B=2, S=2048, D=1024, H=16, dh=64) on 8 TRN2 NeuronCores.
# Sharding: core = batch * 4 + head_group; each core handles one batch and 4 heads.
# v3: k-strip software pipeline. Prologue computes Q/K/V projections for tile 0
# only; per k-strip t the kernel computes scores+exp+PV for every q-tile j >= t
# over that strip's 4 k-tiles, accumulating unnormalized attention and softmax
# denominators (ones-augmented V, M=65) in SBUF across strips. Remaining Q
# projections, K/V projections for strip t+1 and Wo output chunks are
# interleaved as PE filler between the exp-dependent PV matmuls, keeping
# TensorE continuously busy (HAM stays at 8/8) while ScalarE's exp stream
# hides underneath. Causal masking needs only one shared triangular [128,128]
# block applied to the boundary column-block of diagonal tiles (DVE, bf16 2x).
import numpy as np
import ml_dtypes

from collections import deque

import concourse.bass as bass
import concourse.tile as tile
from concourse import bacc, mybir
from concourse import bass_utils

B, S, D = 2, 2048, 1024
H, DH = 16, 64
NCORES = 8
GROUPS = 4            # head groups per batch (cores per batch)
HPG = 4               # heads per group
FPG = HPG * DH        # 256 features per group
SQ_T, SK_T = 512, 128
NSQ, NSK = S // SQ_T, S // SK_T
NCH = D // 128        # 8 contraction chunks of d_model
BF16 = ml_dtypes.bfloat16

_BUILT = {}


def _keep_block(mask):
    """Verify causal structure; the only masking needed is one shared
    upper-triangular [128,128] block on the boundary column-block of each
    diagonal tile. Tile (i, j): k in [i*128, (i+1)*128), q-tile j of 512."""
    keep_t = (~np.asarray(mask, dtype=bool)).T  # [k, q], True = attend
    blk = np.triu(np.ones((SK_T, SK_T), np.float32)).astype(BF16)
    for j in range(NSQ):
        for i in range(NSK):
            sub = keep_t[i * SK_T:(i + 1) * SK_T, j * SQ_T:(j + 1) * SQ_T]
            if i >= 4 * (j + 1):
                assert not sub.any()
            elif i // 4 == j:  # diagonal tile: boundary block + kept tail
                c0 = (i % 4) * SK_T
                assert (sub[:, c0:c0 + SK_T] == (blk != 0)).all()
                assert sub[:, c0 + SK_T:].all()
                assert not sub[:, :c0].any()
            else:
                assert sub.all()
    return blk


def _build():
    nc = bacc.Bacc("TRN2", target_bir_lowering=False, debug=False)
    dt = mybir.dt
    f32, bf = dt.float32, dt.bfloat16
    EXP = mybir.ActivationFunctionType.Exp
    MUL = mybir.AluOpType.mult
    ADD = mybir.AluOpType.add

    xq = nc.dram_tensor("xqt", [D, S], bf, kind="ExternalInput").ap()
    xk = nc.dram_tensor("xkt", [D, S], bf, kind="ExternalInput").ap()
    xv = nc.dram_tensor("xvt", [D, S], bf, kind="ExternalInput").ap()
    wq = nc.dram_tensor("wqt", [D, FPG], bf, kind="ExternalInput").ap()
    wk = nc.dram_tensor("wkt", [D, FPG], bf, kind="ExternalInput").ap()
    wv = nc.dram_tensor("wvt", [D, FPG], bf, kind="ExternalInput").ap()
    wo = nc.dram_tensor("wot", [FPG, D], bf, kind="ExternalInput").ap()
    kp = nc.dram_tensor("keep", [SK_T, SK_T], bf, kind="ExternalInput").ap()
    out = nc.dram_tensor("out", [S, D], f32, kind="ExternalOutput").ap()

    xq_v = xq.rearrange("(c p) s -> p c s", p=128)
    xk_v = xk.rearrange("(c p) s -> p c s", p=128)
    xv_v = xv.rearrange("(c p) s -> p c s", p=128)
    wq_v = wq.rearrange("(c p) f -> p c f", p=128)
    wk_v = wk.rearrange("(c p) f -> p c f", p=128)
    wv_v = wv.rearrange("(c p) f -> p c f", p=128)
    wo_v = wo.rearrange("(c p) o -> p c o", p=128)
    out_v = out.rearrange("(r p) o -> r p o", p=128)

    with tile.TileContext(nc) as tc:
        with (
            tc.tile_pool(name="consts", bufs=1) as consts,
            tc.tile_pool(name="x", bufs=2) as xpool,
            tc.tile_pool(name="sc", bufs=2, space="PSUM") as sc_ps,
            tc.tile_pool(name="pv", bufs=2, space="PSUM") as pv_ps,
            tc.tile_pool(name="aux", bufs=2, space="PSUM") as aux_ps,
            tc.tile_pool(name="work", bufs=4) as work,
            tc.tile_pool(name="probs", bufs=6) as prpool,
        ):
            # ---------------- SBUF constants ----------------
            wq_sb = consts.tile([128, NCH, FPG], bf)
            wk_sb = consts.tile([128, NCH, FPG], bf)
            wv_sb = consts.tile([128, NCH, FPG], bf)
            wo_sb = consts.tile([128, FPG // 128, D], bf)
            keep_sb = consts.tile([128, SK_T], bf)
            ones_sb = consts.tile([128, 128], bf)
            xq_sb = consts.tile([128, NCH, S], bf)       # full Q input
            qh_sb = consts.tile([128, 2, S], bf)
            kh_sb = consts.tile([128, 2, S], bf)
            vh_sb = consts.tile([128, NSK, HPG, DH + 1], bf)
            att_acc = consts.tile([128, 2, S], f32)      # unnormalized att
            att_sb = consts.tile([128, 2, S], bf)        # normalized (Wo in)
            l_acc = consts.tile([128, NSQ, SQ_T], f32)   # denominators
            r4f = consts.tile([128, SQ_T], f32)
            r4 = consts.tile([128, SQ_T], bf)

            # ------- initial DMAs: first working set leads each queue -----
            # sync queue: Q-side; scalar queue: K/V-side
            nc.sync.dma_start(wq_sb[:, 0:4, :], wq_v[:, 0:4, :])
            nc.sync.dma_start(wq_sb[:, 4:8, :], wq_v[:, 4:8, :])
            nc.sync.dma_start(xq_sb[:, 0:4, 0:SQ_T], xq_v[:, 0:4, 0:SQ_T])
            nc.sync.dma_start(xq_sb[:, 4:8, 0:SQ_T], xq_v[:, 4:8, 0:SQ_T])
            nc.scalar.dma_start(wk_sb[:, 0:4, :], wk_v[:, 0:4, :])
            nc.scalar.dma_start(wk_sb[:, 4:8, :], wk_v[:, 4:8, :])
            xk0 = xpool.tile([128, NCH, SQ_T], bf, tag="xk")
            nc.scalar.dma_start(xk0[:, 0:4, :], xk_v[:, 0:4, 0:SQ_T])
            nc.scalar.dma_start(xk0[:, 4:8, :], xk_v[:, 4:8, 0:SQ_T])
            nc.scalar.dma_start(keep_sb[:], kp)
            nc.scalar.dma_start(wv_sb[:], wv_v[:])
            xv0 = xpool.tile([128, NCH, SQ_T], bf, tag="xv")
            nc.scalar.dma_start(xv0[:, 0:4, :], xv_v[:, 0:4, 0:SQ_T])
            nc.scalar.dma_start(xv0[:, 4:8, :], xv_v[:, 4:8, 0:SQ_T])
            xk_t, xv_t = {0: xk0}, {0: xv0}
            for t in range(1, NSQ):
                nc.sync.dma_start(xq_sb[:, :, bass.ts(t, SQ_T)],
                                  xq_v[:, :, bass.ts(t, SQ_T)])
            nc.sync.dma_start(wo_sb[:], wo_v[:])

            nc.vector.memset(ones_sb[:], 1.0)
            nc.gpsimd.memset(vh_sb[:], 1.0)  # ones column (col DH) survives
            nc.gpsimd.memset(l_acc[:], 1.0)  # untouched partitions -> r = 1
            # warm up the ACT exp table under the projection phase
            warm = work.tile([1, 8], f32, tag="warm")
            nc.scalar.activation(warm[:], ones_sb[0:1, 0:8], EXP, scale=0.0)

            def dma_kv(t):
                sl = bass.ts(t, SQ_T)
                xkt = xpool.tile([128, NCH, SQ_T], bf, tag="xk")
                nc.scalar.dma_start(xkt[:], xk_v[:, :, sl])
                xvt = xpool.tile([128, NCH, SQ_T], bf, tag="xv")
                nc.scalar.dma_start(xvt[:], xv_v[:, :, sl])
                xk_t[t], xv_t[t] = xkt, xvt

            # ---------------- projection closures ----------------
            def qk_closures(t, wsb, xin_f, hout, full_x):
                """4 closures: (hp=0 c0-3, hp=0 c4-7+evac, hp=1 ...).
                full_x: xin_f() spans all of S (slice by t); else one tile."""
                sl = bass.ts(t, SQ_T)
                cl = []
                for hp in range(2):
                    hsl = bass.ts(hp, 128)
                    box = {}

                    def xsl(xin, c):
                        return xin[:, c, sl] if full_x else xin[:, c, :]

                    def first(hsl=hsl, box=box, wsb=wsb, xin_f=xin_f,
                              xsl=xsl):
                        ps = aux_ps.tile([128, SQ_T], f32, tag="aux")
                        xin = xin_f()
                        for c in range(4):
                            nc.tensor.matmul(ps[:], wsb[:, c, hsl],
                                             xsl(xin, c),
                                             start=(c == 0), stop=False)
                        box["ps"] = ps

                    def second(hp=hp, hsl=hsl, box=box, sl=sl, wsb=wsb,
                               xin_f=xin_f, hout=hout, xsl=xsl):
                        ps = box["ps"]
                        xin = xin_f()
                        for c in range(4, NCH):
                            nc.tensor.matmul(ps[:], wsb[:, c, hsl],
                                             xsl(xin, c),
                                             start=False, stop=(c == NCH - 1))
                        nc.vector.tensor_copy(hout[:, hp, sl], ps[:])

                    cl += [first, second]
                return cl

            def v_closures(t):
                """8 closures: per s4 subtile (c0-3, c4-7+evac)."""
                cl = []
                for s4 in range(SQ_T // SK_T):
                    i = t * (SQ_T // SK_T) + s4
                    box = {}

                    def vfirst(s4=s4, box=box, t=t):
                        ps = aux_ps.tile([128, SQ_T], f32, tag="aux")
                        for c in range(4):
                            nc.tensor.matmul(ps[:, 0:FPG],
                                             xv_t[t][:, c, bass.ts(s4, SK_T)],
                                             wv_sb[:, c, :],
                                             start=(c == 0), stop=False)
                        box["ps"] = ps

                    def vsecond(s4=s4, box=box, t=t, i=i):
                        ps = box["ps"]
                        for c in range(4, NCH):
                            nc.tensor.matmul(ps[:, 0:FPG],
                                             xv_t[t][:, c, bass.ts(s4, SK_T)],
                                             wv_sb[:, c, :],
                                             start=False, stop=(c == NCH - 1))
                        nc.vector.tensor_copy(
                            vh_sb[:, i, :, 0:DH],
                            ps[:, 0:FPG].rearrange("p (h d) -> p h d", h=HPG))

                    cl += [vfirst, vsecond]
                return cl

            def wo_closures(j):
                """8 closures: per (t4, o) a 2-MM accumulation + evac + DMA."""
                cl = []
                for t4 in range(SQ_T // 128):
                    r_ = j * (SQ_T // 128) + t4
                    tsl = bass.ds(j * SQ_T + t4 * 128, 128)
                    for o in range(2):
                        def wone(r_=r_, tsl=tsl, o=o):
                            po = aux_ps.tile([128, SQ_T], f32, tag="aux")
                            for hp in range(2):
                                nc.tensor.matmul(po[:], att_sb[:, hp, tsl],
                                                 wo_sb[:, hp, bass.ts(o, 512)],
                                                 start=(hp == 0),
                                                 stop=(hp == 1))
                            ost = work.tile([128, 512], f32, tag="ost")
                            nc.vector.tensor_copy(ost[:], po[:])
                            nc.sync.dma_start(out_v[r_, :, bass.ts(o, 512)],
                                              ost[:])
                        cl.append(wone)
                return cl

            # ---------------- filler machinery ----------------
            fillers = deque()  # (tag, closure)

            def drain_tag(tag):
                keep = deque()
                while fillers:
                    tg, fn = fillers.popleft()
                    if tg == tag:
                        fn()
                    else:
                        keep.append((tg, fn))
                fillers.extend(keep)

            def pop_fillers(n):
                for _ in range(n):
                    if not fillers:
                        return
                    _, fn = fillers.popleft()
                    fn()

            # ------- prologue: tile-0 projections only ------------
            for fn in qk_closures(0, wq_sb, lambda: xq_sb, qh_sb, True):
                fn()
            for fn in qk_closures(0, wk_sb, lambda: xk_t[0], kh_sb, False):
                fn()
            for fn in v_closures(0):
                fn()
            for t in range(1, NSQ):
                for fn in qk_closures(t, wq_sb, lambda: xq_sb, qh_sb, True):
                    fillers.append((("q", t), fn))

            # ---------------- k-strips ----------------
            for t in range(NSQ):
                if t + 1 < NSQ:
                    dma_kv(t + 1)
                    for fn in qk_closures(t + 1, wk_sb,
                                          (lambda tt: lambda: xk_t[tt])(t + 1),
                                          kh_sb, False):
                        fillers.append((("kv", t + 1), fn))
                    for fn in v_closures(t + 1):
                        fillers.append((("kv", t + 1), fn))
                drain_tag(("kv", t))  # ensure this strip's K/V emitted
                positions = (NSQ - t) * 2 * 4
                for j in range(t, NSQ):
                    drain_tag(("q", j))  # q-tile j projection must be emitted
                    jsl = bass.ts(j, SQ_T)
                    diag = (j == t)
                    for hp in range(2):
                        pv0 = pv_ps.tile([DH + 1, SQ_T], f32, tag="pv")
                        pv1 = pv_ps.tile([DH + 1, SQ_T], f32, tag="pv")
                        for n in range(4):
                            i = 4 * t + n
                            isl = bass.ts(i, SK_T)
                            c0 = n * SK_T if diag else 0
                            qsl = bass.ds(j * SQ_T + c0, SQ_T - c0)
                            sc = sc_ps.tile([128, 2, SQ_T], f32, tag="sc")
                            nc.tensor.matmul(sc[:, 0, c0:SQ_T],
                                             kh_sb[0:64, hp, isl],
                                             qh_sb[0:64, hp, qsl], start=True,
                                             stop=True, tile_position=(0, 0))
                            nc.tensor.matmul(sc[:, 1, c0:SQ_T],
                                             kh_sb[64:128, hp, isl],
                                             qh_sb[64:128, hp, qsl],
                                             start=True, stop=True,
                                             tile_position=(64, 0))
                            pr = prpool.tile([128, 2, SQ_T], bf, tag="probs")
                            nc.scalar.activation(pr[:, :, c0:SQ_T],
                                                 sc[:, :, c0:SQ_T],
                                                 EXP, scale=0.125)
                            if diag:
                                # mask only the triangular boundary block
                                nc.vector.tensor_mul(pr[:, 0, c0:c0 + SK_T],
                                                     pr[:, 0, c0:c0 + SK_T],
                                                     keep_sb[:])
                                nc.vector.tensor_mul(pr[:, 1, c0:c0 + SK_T],
                                                     pr[:, 1, c0:c0 + SK_T],
                                                     keep_sb[:])
                            # paced PE filler between exp and its PV consumer
                            nfill = -(-len(fillers) // max(positions, 1))
                            pop_fillers(min(nfill, 2))
                            positions -= 1
                            nc.tensor.matmul(pv0[:, c0:SQ_T],
                                             vh_sb[:, i, 2 * hp + 0, :],
                                             pr[:, 0, c0:SQ_T],
                                             start=(n == 0), stop=(n == 3))
                            nc.tensor.matmul(pv1[:, c0:SQ_T],
                                             vh_sb[:, i, 2 * hp + 1, :],
                                             pr[:, 1, c0:SQ_T],
                                             start=(n == 0), stop=(n == 3))
                        # evacuate psum: accumulate att + denominators in SBUF
                        p0 = 64 * hp
                        if t == 0:
                            nc.vector.tensor_copy(att_acc[0:64, hp, jsl],
                                                  pv0[0:64, :])
                            nc.vector.tensor_copy(l_acc[p0:p0 + 1, j, :],
                                                  pv0[DH:DH + 1, :])
                            nc.vector.tensor_copy(att_acc[64:128, hp, jsl],
                                                  pv1[0:64, :])
                            nc.vector.tensor_copy(l_acc[p0 + 32:p0 + 33, j, :],
                                                  pv1[DH:DH + 1, :])
                        else:
                            nc.vector.tensor_tensor(att_acc[0:64, hp, jsl],
                                                    pv0[0:64, :],
                                                    att_acc[0:64, hp, jsl],
                                                    ADD)
                            nc.vector.tensor_tensor(l_acc[p0:p0 + 1, j, :],
                                                    pv0[DH:DH + 1, :],
                                                    l_acc[p0:p0 + 1, j, :],
                                                    ADD)
                            nc.vector.tensor_tensor(att_acc[64:128, hp, jsl],
                                                    pv1[0:64, :],
                                                    att_acc[64:128, hp, jsl],
                                                    ADD)
                            nc.vector.tensor_tensor(
                                l_acc[p0 + 32:p0 + 33, j, :],
                                pv1[DH:DH + 1, :],
                                l_acc[p0 + 32:p0 + 33, j, :], ADD)
                    if diag:
                        # j is complete: normalize and queue its Wo chunks
                        nc.vector.reciprocal_approx_fast(r4f[:],
                                                         l_acc[:, j, :])
                        nc.vector.tensor_copy(r4[:], r4f[:])
                        for hp in range(2):
                            rb = aux_ps.tile([128, SQ_T], f32, tag="aux")
                            pe, po_ = 64 * hp, 64 * hp + 32
                            nc.tensor.matmul(rb[0:64, :],
                                             ones_sb[pe:pe + 1, 0:64],
                                             r4[pe:pe + 1, :], start=True,
                                             stop=True, tile_position=(pe, 0))
                            nc.tensor.matmul(rb[64:128, :],
                                             ones_sb[po_:po_ + 1, 64:128],
                                             r4[po_:po_ + 1, :], start=True,
                                             stop=True,
                                             tile_position=(po_, 64))
                            nc.vector.tensor_tensor(att_sb[:, hp, jsl],
                                                    att_acc[:, hp, jsl],
                                                    rb[:], MUL)
                        for fn in wo_closures(j):
                            fillers.append((("wo", j), fn))
            while fillers:
                _, fn = fillers.popleft()
                fn()

    nc.compile()
    return nc


def _get_nc(mask):
    key = hash(np.asarray(mask, dtype=bool).tobytes())
    if key not in _BUILT:
        blk = _keep_block(mask)
        _BUILT[key] = (_build(), blk)
    return _BUILT[key]


def _kernel_impl(q, k, v, attn_mask, Wq, Wk, Wv, Wo, trace=False):
    q = np.asarray(q, dtype=np.float32)
    k = np.asarray(k, dtype=np.float32)
    v = np.asarray(v, dtype=np.float32)
    nc, keep_blk = _get_nc(attn_mask)

    xt = {}
    for b in range(B):
        xt[("q", b)] = np.ascontiguousarray(q[b].T.astype(BF16))
        xt[("k", b)] = np.ascontiguousarray(k[b].T.astype(BF16))
        xt[("v", b)] = np.ascontiguousarray(v[b].T.astype(BF16))
    wslices = {}
    for g in range(GROUPS):
        fsl = slice(g * FPG, (g + 1) * FPG)
        wslices[("wq", g)] = np.ascontiguousarray(Wq[fsl, :].T.astype(BF16))
        wslices[("wk", g)] = np.ascontiguousarray(Wk[fsl, :].T.astype(BF16))
        wslices[("wv", g)] = np.ascontiguousarray(Wv[fsl, :].T.astype(BF16))
        wslices[("wo", g)] = np.ascontiguousarray(Wo[:, fsl].T.astype(BF16))

    in_maps = []
    for core in range(NCORES):
        b, g = core // GROUPS, core % GROUPS
        in_maps.append({
            "xqt": xt[("q", b)], "xkt": xt[("k", b)], "xvt": xt[("v", b)],
            "wqt": wslices[("wq", g)], "wkt": wslices[("wk", g)],
            "wvt": wslices[("wv", g)], "wot": wslices[("wo", g)],
            "keep": keep_blk,
        })

    res = bass_utils.run_bass_kernel_spmd(
        nc, in_maps, core_ids=list(range(NCORES)), trace=trace)

    out = np.zeros((B, S, D), dtype=np.float32)
    for core in range(NCORES):
        out[core // GROUPS] += res.results[core]["out"]
    return out, res


def kernel(q, k, v, attn_mask, Wq, Wk, Wv, Wo):
    out, _ = _kernel_impl(q, k, v, attn_mask, Wq, Wk, Wv, Wo)
    return out
